# revision 5
# baseline (speedup 1.0000x reference)
"""MoE expert-MLP kernel for Trainium2, expert-parallel across 8 NeuronCores.

Problem: T=8192 tokens, H=1024 hidden, I=4096 intermediate, E=8 experts,
top-K=2, capacity C = T*K/E = 2048 slots per expert.

Strategy:
  - Host (numpy): replicate the reference routing exactly (capacity drop via
    token-order cumsum, top-k affinity renormalization), gather each expert's
    tokens into a dense (C, H) block, transpose to (H, C) so the contraction
    dim lands on SBUF partitions, and pre-tile the weights into lhsT layout.
  - Device (1 expert per core, same SPMD program): two grouped matmuls with
    fused SiLU-GLU, all in bf16 (fp32 PSUM accumulation).  bf16 weights get
    the fast-weight-load path (LDWEIGHTS fully hidden under the matmul),
    unlike fp32r whose 224ns weight loads cost ~13ns per matmul.  The
    per-slot combine scale is folded into the PSUM->SBUF evacuation of y^T
    (exact: scale commutes with the linear matmul).
  - Host: scatter-add y back to token order via the routing permutation.

C is processed in two halves of 1024 so h^T (bf16) fits in SBUF; W1 is
streamed twice (bf16: 2x16MB) which hides under the ~800us of PE work.
x^T lives in one 3D SBUF tile [128, KH, C] loaded chunk-major with few
large DMAs; the first k-tile of chunk 0 is a separate small DMA so the
very first matmul's dependencies land as early as possible (the sync
engine issues DMA descriptors at ~0.7us each, so issue ORDER, not
bandwidth, dominates time-to-first-matmul).
"""

import sys

import numpy as np

try:
    import concourse.bass as bass  # noqa: F401
except ImportError:
    for p in ("/opt/trn_rl_repo", "/root/.axon_site/_ro/trn_rl_repo"):
        if p not in sys.path:
            sys.path.insert(0, p)

import ml_dtypes
import concourse.bass as bass
import concourse.tile as tile
from concourse import bacc, mybir
from concourse.bass_utils import run_bass_kernel_spmd

dt = mybir.dt

T, H, I, E, K = 8192, 1024, 4096, 8, 2
C = 2048  # min(T, ceil(T*K*1.0/E))
KH = H // 128   # 8  k-tiles for the first contraction
KI = I // 128   # 32 k-tiles for the second contraction
M1 = 2 * I // 128  # 64 output tiles of gu^T
M2 = H // 128   # 8  output tiles of y^T
NCH = 2         # C halves
NW = 2          # 512-wide chunks per half
NQ = 4          # 512-wide chunks of C for x loading

_CACHED_NC = None


def _build_nc():
    nc = bacc.Bacc(None)
    xt = nc.dram_tensor("xt", [NQ, 128, KH, 512], dt.bfloat16, kind="ExternalInput")
    # w1 pairs the gate tile m and up tile m+M1/2 in one [128, 2048] row so a
    # single DMA fetches both per m-step.
    w1 = nc.dram_tensor("w1", [M1 // 2, 128, 2 * KH * 128], dt.bfloat16,
                        kind="ExternalInput")
    w2 = nc.dram_tensor("w2", [M2, 128, KI * 128], dt.bfloat16, kind="ExternalInput")
    sc = nc.dram_tensor("sc", [128, C], dt.float32, kind="ExternalInput")
    yt = nc.dram_tensor("yt", [M2, 128, C], dt.float32, kind="ExternalOutput")

    with tile.TileContext(nc) as tc:
        with (
            tc.tile_pool(name="xt_p", bufs=1) as xt_p,
            tc.tile_pool(name="sc_p", bufs=1) as sc_p,
            tc.tile_pool(name="ht_p", bufs=1) as ht_p,
            tc.tile_pool(name="w1_p", bufs=2) as w1_p,
            tc.tile_pool(name="w2_p", bufs=4) as w2_p,
            tc.tile_pool(name="sg_p", bufs=4) as sg_p,
            tc.tile_pool(name="yb_p", bufs=6) as yb_p,
            tc.tile_pool(name="ps", bufs=2, space="PSUM") as ps,
            tc.tile_pool(name="ps_y", bufs=4, space="PSUM") as ps_y,
        ):
            xt_t = xt_p.tile([128, KH, C], dt.bfloat16, tag="xt", name="xt_t")
            sc_t = sc_p.tile([128, C], dt.float32, tag="sc")

            def load_xt_chunk(q):
                nc.sync.dma_start(xt_t[:, :, q * 512:(q + 1) * 512], xt[q])

            # Head: two HWDGE issue streams in parallel.  Sync carries the x
            # chunk (k0/k1 first so matmul 0/1 can start, then the rest);
            # Scalar carries the first weight pair.  The sync engine needs
            # ~0.65us to issue each DMA, so issue order — not bandwidth —
            # sets time-to-first-matmul.
            w1gu0 = w1_p.tile([128, 2 * KH * 128], dt.bfloat16, tag="w1gu")
            nc.scalar.dma_start(w1gu0[:, 0:KH * 128], w1[0][:, 0:KH * 128])
            nc.scalar.dma_start(w1gu0[:, KH * 128:], w1[0][:, KH * 128:])
            nc.sync.dma_start(xt_t[:, 0:2, 0:512], xt[0][:, 0:2, :])
            nc.sync.dma_start(xt_t[:, 2:, 0:512], xt[0][:, 2:, :])
            load_xt_chunk(1)

            for ch in range(NCH):
                cw = C // NCH  # 1024
                # ---- phase 1: gu^T = W1^T x^T ; h^T = silu(g)*u (bf16) ----
                ht_t = []
                for mh in range(M1 // 2):
                    if ch == 0 and mh == 4:
                        load_xt_chunk(2)
                    if ch == 0 and mh == 8:
                        load_xt_chunk(3)
                    if ch == 0 and mh == 12:
                        nc.sync.dma_start(sc_t[:], sc[:])
                    if ch == 0 and mh == 0:
                        w1gu = w1gu0
                    else:
                        w1gu = w1_p.tile([128, 2 * KH * 128], dt.bfloat16,
                                         tag="w1gu")
                        nc.scalar.dma_start(w1gu[:], w1[mh])
                    w1g = w1gu[:, 0:KH * 128]
                    w1u = w1gu[:, KH * 128:]
                    ht = ht_p.tile([128, cw], dt.bfloat16, tag=f"ht{mh}")
                    for n in range(NW):
                        col = ch * cw + n * 512
                        pg = ps.tile([128, 512], dt.float32, tag="pg")
                        pu = ps.tile([128, 512], dt.float32, tag="pu")
                        for k in range(KH):
                            nc.tensor.matmul(
                                pg[:],
                                w1g[:, k * 128:(k + 1) * 128],
                                xt_t[:, k, col:col + 512],
                                start=(k == 0),
                                stop=(k == KH - 1),
                            )
                        for k in range(KH):
                            nc.tensor.matmul(
                                pu[:],
                                w1u[:, k * 128:(k + 1) * 128],
                                xt_t[:, k, col:col + 512],
                                start=(k == 0),
                                stop=(k == KH - 1),
                            )
                        sg = sg_p.tile([128, 512], dt.float32, tag="sg")
                        nc.scalar.activation(
                            sg[:], pg[:], mybir.ActivationFunctionType.Silu
                        )
                        nc.vector.tensor_mul(
                            ht[:, n * 512:(n + 1) * 512], sg[:], pu[:]
                        )
                    ht_t.append(ht)

                # ---- phase 2: y^T = W2^T h^T, scaled on evacuation ----
                for m in range(M2):
                    w2t = w2_p.tile([128, KI * 128], dt.bfloat16, tag="w2")
                    nc.sync.dma_start(w2t[:], w2[m])
                    for n in range(NW):
                        col = ch * cw + n * 512
                        py = ps_y.tile([128, 512], dt.float32, tag="py")
                        for k in range(KI):
                            nc.tensor.matmul(
                                py[:],
                                w2t[:, k * 128:(k + 1) * 128],
                                ht_t[k][:, n * 512:(n + 1) * 512],
                                start=(k == 0),
                                stop=(k == KI - 1),
                            )
                        yb = yb_p.tile([128, 512], dt.float32, tag="yb")
                        nc.vector.tensor_mul(yb[:], py[:], sc_t[:, col:col + 512])
                        nc.sync.dma_start(yt[m][:, col:col + 512], yb[:])
    nc.finalize()
    return nc


def _route(expert_affinities, expert_index):
    """Numpy port of the reference routing. Returns (tok_idx, valid, scale)."""
    mask = np.zeros((T, E), dtype=np.float32)
    rows = np.arange(T)[:, None]
    mask[rows, expert_index] = 1.0  # top-k entries are distinct per token
    position = np.cumsum(mask, axis=0, dtype=np.float32)  # 1-based
    mask = np.where(position > C, 0.0, mask)

    affin = np.where(mask == 0, 0.0, expert_affinities)
    denom = np.maximum(np.sum(np.abs(affin), axis=1, keepdims=True), 1e-12)
    affin = affin / denom

    offsets = np.arange(E, dtype=np.float32) * C
    pos_off = np.where(mask == 0, 0.0, position + offsets)
    perm_idx = np.take_along_axis(pos_off, expert_index, axis=1).astype(np.int32)

    tok_ids = np.broadcast_to(
        np.arange(1, T + 1, dtype=np.int32)[:, None], (T, K)
    )
    assignments = np.zeros(E * C + 1, dtype=np.int32)
    assignments[perm_idx.reshape(-1)] = tok_ids.reshape(-1)
    assignments = assignments[1:].reshape(E, C) - 1
    valid = assignments >= 0
    tok_idx = np.maximum(assignments, 0)

    scale = affin[tok_idx, np.arange(E)[:, None]] * valid.astype(np.float32)
    return tok_idx, valid, scale, perm_idx


def prepare_in_maps(hidden_states, expert_affinities, expert_index,
                    gate_up_proj, down_proj):
    hidden_states = np.asarray(hidden_states, dtype=np.float32)
    expert_affinities = np.asarray(expert_affinities, dtype=np.float32)
    expert_index = np.asarray(expert_index, dtype=np.int32)
    gate_up_proj = np.asarray(gate_up_proj, dtype=np.float32)
    down_proj = np.asarray(down_proj, dtype=np.float32)

    tok_idx, valid, scale, perm_idx = _route(expert_affinities, expert_index)

    def _prep_expert(e):
        x_e = hidden_states[tok_idx[e]]  # (C, H)
        xt_e = np.ascontiguousarray(x_e.T).astype(ml_dtypes.bfloat16)  # (H, C)
        xt_e = np.ascontiguousarray(
            xt_e.reshape(KH, 128, NQ, 512).transpose(2, 1, 0, 3)
        )  # (NQ, 128, KH, 512)
        # (m, p, pair, k, j): pair 0 = gate column block m, pair 1 = up block m
        w1_e = np.ascontiguousarray(
            gate_up_proj[e].astype(ml_dtypes.bfloat16)
            .reshape(KH, 128, 2, M1 // 2, 128).transpose(3, 1, 2, 0, 4)
        ).reshape(M1 // 2, 128, 2 * KH * 128)
        w2_e = np.ascontiguousarray(
            down_proj[e].astype(ml_dtypes.bfloat16)
            .reshape(KI, 128, M2, 128).transpose(2, 1, 0, 3)
        ).reshape(M2, 128, KI * 128)
        sc_e = np.broadcast_to(scale[e][None, :], (128, C)).copy()
        return {"xt": xt_e, "w1": w1_e, "w2": w2_e, "sc": sc_e}

    from concurrent.futures import ThreadPoolExecutor
    with ThreadPoolExecutor(max_workers=E) as pool:
        in_maps = list(pool.map(_prep_expert, range(E)))
    return in_maps, perm_idx


def run_spmd(in_maps, **kwargs):
    global _CACHED_NC
    if _CACHED_NC is None:
        _CACHED_NC = _build_nc()
    return run_bass_kernel_spmd(
        _CACHED_NC, in_maps, core_ids=list(range(E)), **kwargs
    )


_CACHED_RUNNER = None


def _fast_run(in_maps):
    """Same semantics as run_bass_kernel_spmd under axon, but the jitted
    shard_map callable is built once and reused, avoiding per-call retrace."""
    global _CACHED_NC, _CACHED_RUNNER
    if _CACHED_RUNNER is None:
        if _CACHED_NC is None:
            _CACHED_NC = _build_nc()
        nc = _CACHED_NC
        import jax
        from jax.sharding import Mesh, PartitionSpec
        try:
            from jax.experimental.shard_map import shard_map
        except ImportError:
            from jax.shard_map import shard_map  # newer jax
        from concourse import bass2jax, mybir as _mybir
        bass2jax.install_neuronx_cc_hook()

        partition_name = (
            nc.partition_id_tensor.name if nc.partition_id_tensor else None
        )
        in_names, out_names, out_avals = [], [], []
        for alloc in nc.m.functions[0].allocations:
            if not isinstance(alloc, _mybir.MemoryLocationSet):
                continue
            name = alloc.memorylocations[0].name
            if alloc.kind == "ExternalInput":
                if name != partition_name:
                    in_names.append(name)
            elif alloc.kind == "ExternalOutput":
                out_names.append(name)
                out_avals.append(jax.core.ShapedArray(
                    tuple(alloc.tensor_shape), _mybir.dt.np(alloc.dtype)))
        n_params = len(in_names)
        n_outs = len(out_avals)
        all_in_names = list(in_names) + list(out_names)
        if partition_name is not None:
            all_in_names.append(partition_name)
        donate = tuple(range(n_params, n_params + n_outs))

        def _body(*args):
            operands = list(args)
            if partition_name is not None:
                operands.append(bass2jax.partition_id_tensor())
            outs = bass2jax._bass_exec_p.bind(
                *operands,
                out_avals=tuple(out_avals),
                in_names=tuple(all_in_names),
                out_names=tuple(out_names),
                lowering_input_output_aliases=(),
                sim_require_finite=True,
                sim_require_nnan=True,
                nc=nc,
            )
            return tuple(outs)

        devices = jax.devices()[:E]
        mesh = Mesh(np.array(devices), ("core",))
        in_specs = (PartitionSpec("core"),) * (n_params + n_outs)
        out_specs = (PartitionSpec("core"),) * n_outs
        sharded = jax.jit(
            shard_map(_body, mesh=mesh, in_specs=in_specs,
                      out_specs=out_specs, check_rep=False),
            donate_argnums=donate, keep_unused=True,
        )
        _CACHED_RUNNER = (sharded, in_names, out_names, out_avals)

    sharded, in_names, out_names, out_avals = _CACHED_RUNNER
    concat_in = [
        np.concatenate([np.asarray(m[name]) for m in in_maps], axis=0)
        for name in in_names
    ]
    concat_zeros = [
        np.zeros((E * a.shape[0], *a.shape[1:]), a.dtype) for a in out_avals
    ]
    out_arrs = sharded(*concat_in, *concat_zeros)
    results = [
        {name: np.asarray(out_arrs[i]).reshape(E, *out_avals[i].shape)[c]
         for i, name in enumerate(out_names)}
        for c in range(E)
    ]
    return results


def combine(res, perm_idx):
    # (E, C, H) of scaled expert outputs, then scatter-add back to tokens.
    y_flat = np.empty((E * C, H), dtype=np.float32)
    for e in range(E):
        yt_e = res.results[e]["yt"].reshape(H, C)  # (H, C)
        y_flat[e * C:(e + 1) * C] = yt_e.T
    out = np.zeros((T, H), dtype=np.float32)
    for k in range(K):
        idx = perm_idx[:, k]
        m = idx > 0
        out[m] += y_flat[idx[m] - 1]
    return out


def kernel(hidden_states, expert_affinities, expert_index, gate_up_proj, down_proj):
    in_maps, perm_idx = prepare_in_maps(
        hidden_states, expert_affinities, expert_index, gate_up_proj, down_proj
    )
    try:
        results = _fast_run(in_maps)
    except Exception:
        results = run_spmd(in_maps).results
    return combine(_Res(results), perm_idx)


class _Res:
    def __init__(self, results):
        self.results = results


# revision 8
# speedup vs baseline: 1.0007x; 1.0007x over previous
"""MoE expert-MLP kernel for Trainium2, expert-parallel across 8 NeuronCores.

Problem: T=8192 tokens, H=1024 hidden, I=4096 intermediate, E=8 experts,
top-K=2, capacity C = T*K/E = 2048 slots per expert.

Strategy:
  - Host (numpy): replicate the reference routing exactly (capacity drop via
    token-order cumsum, top-k affinity renormalization), gather each expert's
    tokens into a dense (C, H) block, transpose to (H, C) so the contraction
    dim lands on SBUF partitions, and pre-tile the weights into lhsT layout.
  - Device (1 expert per core, same SPMD program): two grouped matmuls with
    fused SiLU-GLU, all in bf16 (fp32 PSUM accumulation).  bf16 weights get
    the fast-weight-load path (LDWEIGHTS fully hidden under the matmul),
    unlike fp32r whose 224ns weight loads cost ~13ns per matmul.  The
    per-slot combine scale is folded into the PSUM->SBUF evacuation of y^T
    (exact: scale commutes with the linear matmul).
  - Host: scatter-add y back to token order via the routing permutation.

C is processed in two halves of 1024 so h^T (bf16) fits in SBUF; W1 is
streamed twice (bf16: 2x16MB) which hides under the ~800us of PE work.
x^T lives in one 3D SBUF tile [128, KH, C] loaded chunk-major with few
large DMAs; the first k-tile of chunk 0 is a separate small DMA so the
very first matmul's dependencies land as early as possible (the sync
engine issues DMA descriptors at ~0.7us each, so issue ORDER, not
bandwidth, dominates time-to-first-matmul).
"""

import sys

import numpy as np

try:
    import concourse.bass as bass  # noqa: F401
except ImportError:
    for p in ("/opt/trn_rl_repo", "/root/.axon_site/_ro/trn_rl_repo"):
        if p not in sys.path:
            sys.path.insert(0, p)

import ml_dtypes
import concourse.bass as bass
import concourse.tile as tile
from concourse import bacc, mybir
from concourse.bass_utils import run_bass_kernel_spmd

dt = mybir.dt

T, H, I, E, K = 8192, 1024, 4096, 8, 2
C = 2048  # min(T, ceil(T*K*1.0/E))
KH = H // 128   # 8  k-tiles for the first contraction
KI = I // 128   # 32 k-tiles for the second contraction
M1 = 2 * I // 128  # 64 output tiles of gu^T
M2 = H // 128   # 8  output tiles of y^T
NCH = 2         # C halves
NW = 2          # 512-wide chunks per half
NQ = 4          # 512-wide chunks of C for x loading

_CACHED_NC = None


def _build_nc():
    nc = bacc.Bacc(None)
    xt = nc.dram_tensor("xt", [NQ, 128, KH, 512], dt.bfloat16, kind="ExternalInput")
    # w1 pairs the gate tile m and up tile m+M1/2 in one [128, 2048] row so a
    # single DMA fetches both per m-step.
    w1 = nc.dram_tensor("w1", [M1 // 2, 128, 2 * KH * 128], dt.bfloat16,
                        kind="ExternalInput")
    w2 = nc.dram_tensor("w2", [M2, 128, KI * 128], dt.bfloat16, kind="ExternalInput")
    sc = nc.dram_tensor("sc", [128, C], dt.float32, kind="ExternalInput")
    yt = nc.dram_tensor("yt", [M2, 128, C], dt.float32, kind="ExternalOutput")

    with tile.TileContext(nc) as tc:
        with (
            tc.tile_pool(name="xt_p", bufs=1) as xt_p,
            tc.tile_pool(name="sc_p", bufs=1) as sc_p,
            tc.tile_pool(name="ht_p", bufs=1) as ht_p,
            tc.tile_pool(name="w1_p", bufs=2) as w1_p,
            tc.tile_pool(name="w2_p", bufs=4) as w2_p,
            tc.tile_pool(name="sg_p", bufs=4) as sg_p,
            tc.tile_pool(name="yb_p", bufs=6) as yb_p,
            tc.tile_pool(name="ps", bufs=2, space="PSUM") as ps,
            tc.tile_pool(name="ps_y", bufs=4, space="PSUM") as ps_y,
        ):
            xt_t = xt_p.tile([128, KH, C], dt.bfloat16, tag="xt", name="xt_t")
            sc_t = sc_p.tile([128, C], dt.float32, tag="sc")

            def load_xt_chunk(q):
                nc.sync.dma_start(xt_t[:, :, q * 512:(q + 1) * 512], xt[q])

            # Head: two HWDGE issue streams in parallel.  Sync carries the x
            # chunk (k0/k1 first so matmul 0/1 can start, then the rest);
            # Scalar carries the first weight pair, sliced so the first
            # matmuls' weights land first.  The issuing engine needs ~0.65us
            # per DMA, so issue order — not bandwidth — sets
            # time-to-first-matmul.
            w1gu0 = w1_p.tile([128, 2 * KH * 128], dt.bfloat16, tag="w1gu",
                              bufs=6)
            nc.scalar.dma_start(w1gu0[:, 0:256], w1[0][:, 0:256])
            nc.scalar.dma_start(w1gu0[:, 256:KH * 128], w1[0][:, 256:KH * 128])
            nc.scalar.dma_start(w1gu0[:, KH * 128:], w1[0][:, KH * 128:])
            nc.sync.dma_start(xt_t[:, 0:2, 0:512], xt[0][:, 0:2, :])
            nc.sync.dma_start(xt_t[:, 2:, 0:512], xt[0][:, 2:, :])
            load_xt_chunk(1)

            # First HEAD_M m-tiles run their n=0 window before any n=1 work:
            # the early DMA pipe can't feed both 512-col x chunks at the rate
            # the PE consumes them, so halve the early x working set.
            HEAD_M = 4

            def phase1_order():
                for mh in range(HEAD_M):
                    yield mh, 0
                for mh in range(HEAD_M):
                    yield mh, 1
                for mh in range(HEAD_M, M1 // 2):
                    yield mh, 0
                    yield mh, 1

            for ch in range(NCH):
                cw = C // NCH  # 1024
                # ---- phase 1: gu^T = W1^T x^T ; h^T = silu(g)*u (bf16) ----
                ht_t = {}
                w1gu_t = {}
                for mh, n in phase1_order():
                    if ch == 0 and n == 0:
                        if mh == 4:
                            load_xt_chunk(2)
                        if mh == 8:
                            load_xt_chunk(3)
                        if mh == 12:
                            nc.sync.dma_start(sc_t[:], sc[:])
                    if n == 0:
                        if ch == 0 and mh == 0:
                            w1gu = w1gu0
                        else:
                            w1gu = w1_p.tile([128, 2 * KH * 128], dt.bfloat16,
                                             tag="w1gu", bufs=6)
                            nc.scalar.dma_start(w1gu[:], w1[mh])
                        w1gu_t[mh] = w1gu
                        ht_t[mh] = ht_p.tile([128, cw], dt.bfloat16,
                                             tag=f"ht{mh}", name=f"ht{mh}")
                    w1gu = w1gu_t[mh]
                    ht = ht_t[mh]
                    w1g = w1gu[:, 0:KH * 128]
                    w1u = w1gu[:, KH * 128:]
                    col = ch * cw + n * 512
                    pg = ps.tile([128, 512], dt.float32, tag="pg")
                    pu = ps.tile([128, 512], dt.float32, tag="pu")
                    for k in range(KH):
                        nc.tensor.matmul(
                            pg[:],
                            w1g[:, k * 128:(k + 1) * 128],
                            xt_t[:, k, col:col + 512],
                            start=(k == 0),
                            stop=(k == KH - 1),
                        )
                    for k in range(KH):
                        nc.tensor.matmul(
                            pu[:],
                            w1u[:, k * 128:(k + 1) * 128],
                            xt_t[:, k, col:col + 512],
                            start=(k == 0),
                            stop=(k == KH - 1),
                        )
                    sg = sg_p.tile([128, 512], dt.float32, tag="sg")
                    nc.scalar.activation(
                        sg[:], pg[:], mybir.ActivationFunctionType.Silu
                    )
                    nc.vector.tensor_mul(
                        ht[:, n * 512:(n + 1) * 512], sg[:], pu[:]
                    )

                # ---- phase 2: y^T = W2^T h^T, scaled on evacuation ----
                for m in range(M2):
                    w2t = w2_p.tile([128, KI * 128], dt.bfloat16, tag="w2")
                    nc.sync.dma_start(w2t[:], w2[m])
                    for n in range(NW):
                        col = ch * cw + n * 512
                        # The very last output tile drains in 256-col halves
                        # so its evacuation+writeback overlaps the final
                        # matmuls instead of all trailing them.
                        nsplit = 2 if (ch == NCH - 1 and m == M2 - 1
                                       and n == NW - 1) else 1
                        w = 512 // nsplit
                        for hf in range(nsplit):
                            c2 = col + hf * w
                            off = n * 512 + hf * w
                            py = ps_y.tile([128, w], dt.float32, tag="py")
                            for k in range(KI):
                                nc.tensor.matmul(
                                    py[:],
                                    w2t[:, k * 128:(k + 1) * 128],
                                    ht_t[k][:, off:off + w],
                                    start=(k == 0),
                                    stop=(k == KI - 1),
                                )
                            yb = yb_p.tile([128, w], dt.float32, tag="yb")
                            nc.vector.tensor_mul(yb[:], py[:],
                                                 sc_t[:, c2:c2 + w])
                            nc.sync.dma_start(yt[m][:, c2:c2 + w], yb[:])
    nc.finalize()
    return nc


def _route(expert_affinities, expert_index):
    """Numpy port of the reference routing. Returns (tok_idx, valid, scale)."""
    mask = np.zeros((T, E), dtype=np.float32)
    rows = np.arange(T)[:, None]
    mask[rows, expert_index] = 1.0  # top-k entries are distinct per token
    position = np.cumsum(mask, axis=0, dtype=np.float32)  # 1-based
    mask = np.where(position > C, 0.0, mask)

    affin = np.where(mask == 0, 0.0, expert_affinities)
    denom = np.maximum(np.sum(np.abs(affin), axis=1, keepdims=True), 1e-12)
    affin = affin / denom

    offsets = np.arange(E, dtype=np.float32) * C
    pos_off = np.where(mask == 0, 0.0, position + offsets)
    perm_idx = np.take_along_axis(pos_off, expert_index, axis=1).astype(np.int32)

    tok_ids = np.broadcast_to(
        np.arange(1, T + 1, dtype=np.int32)[:, None], (T, K)
    )
    assignments = np.zeros(E * C + 1, dtype=np.int32)
    assignments[perm_idx.reshape(-1)] = tok_ids.reshape(-1)
    assignments = assignments[1:].reshape(E, C) - 1
    valid = assignments >= 0
    tok_idx = np.maximum(assignments, 0)

    scale = affin[tok_idx, np.arange(E)[:, None]] * valid.astype(np.float32)
    return tok_idx, valid, scale, perm_idx


def prepare_in_maps(hidden_states, expert_affinities, expert_index,
                    gate_up_proj, down_proj):
    hidden_states = np.asarray(hidden_states, dtype=np.float32)
    expert_affinities = np.asarray(expert_affinities, dtype=np.float32)
    expert_index = np.asarray(expert_index, dtype=np.int32)
    gate_up_proj = np.asarray(gate_up_proj, dtype=np.float32)
    down_proj = np.asarray(down_proj, dtype=np.float32)

    tok_idx, valid, scale, perm_idx = _route(expert_affinities, expert_index)

    def _prep_expert(e):
        x_e = hidden_states[tok_idx[e]]  # (C, H)
        xt_e = np.ascontiguousarray(x_e.T).astype(ml_dtypes.bfloat16)  # (H, C)
        xt_e = np.ascontiguousarray(
            xt_e.reshape(KH, 128, NQ, 512).transpose(2, 1, 0, 3)
        )  # (NQ, 128, KH, 512)
        # (m, p, pair, k, j): pair 0 = gate column block m, pair 1 = up block m
        w1_e = np.ascontiguousarray(
            gate_up_proj[e].astype(ml_dtypes.bfloat16)
            .reshape(KH, 128, 2, M1 // 2, 128).transpose(3, 1, 2, 0, 4)
        ).reshape(M1 // 2, 128, 2 * KH * 128)
        w2_e = np.ascontiguousarray(
            down_proj[e].astype(ml_dtypes.bfloat16)
            .reshape(KI, 128, M2, 128).transpose(2, 1, 0, 3)
        ).reshape(M2, 128, KI * 128)
        sc_e = np.broadcast_to(scale[e][None, :], (128, C)).copy()
        return {"xt": xt_e, "w1": w1_e, "w2": w2_e, "sc": sc_e}

    from concurrent.futures import ThreadPoolExecutor
    with ThreadPoolExecutor(max_workers=E) as pool:
        in_maps = list(pool.map(_prep_expert, range(E)))
    return in_maps, perm_idx


def run_spmd(in_maps, **kwargs):
    global _CACHED_NC
    if _CACHED_NC is None:
        _CACHED_NC = _build_nc()
    return run_bass_kernel_spmd(
        _CACHED_NC, in_maps, core_ids=list(range(E)), **kwargs
    )


_CACHED_RUNNER = None


def _fast_run(in_maps):
    """Same semantics as run_bass_kernel_spmd under axon, but the jitted
    shard_map callable is built once and reused, avoiding per-call retrace."""
    global _CACHED_NC, _CACHED_RUNNER
    if _CACHED_RUNNER is None:
        if _CACHED_NC is None:
            _CACHED_NC = _build_nc()
        nc = _CACHED_NC
        import jax
        from jax.sharding import Mesh, PartitionSpec
        try:
            from jax.experimental.shard_map import shard_map
        except ImportError:
            from jax.shard_map import shard_map  # newer jax
        from concourse import bass2jax, mybir as _mybir
        bass2jax.install_neuronx_cc_hook()

        partition_name = (
            nc.partition_id_tensor.name if nc.partition_id_tensor else None
        )
        in_names, out_names, out_avals = [], [], []
        for alloc in nc.m.functions[0].allocations:
            if not isinstance(alloc, _mybir.MemoryLocationSet):
                continue
            name = alloc.memorylocations[0].name
            if alloc.kind == "ExternalInput":
                if name != partition_name:
                    in_names.append(name)
            elif alloc.kind == "ExternalOutput":
                out_names.append(name)
                out_avals.append(jax.core.ShapedArray(
                    tuple(alloc.tensor_shape), _mybir.dt.np(alloc.dtype)))
        n_params = len(in_names)
        n_outs = len(out_avals)
        all_in_names = list(in_names) + list(out_names)
        if partition_name is not None:
            all_in_names.append(partition_name)
        donate = tuple(range(n_params, n_params + n_outs))

        def _body(*args):
            operands = list(args)
            if partition_name is not None:
                operands.append(bass2jax.partition_id_tensor())
            outs = bass2jax._bass_exec_p.bind(
                *operands,
                out_avals=tuple(out_avals),
                in_names=tuple(all_in_names),
                out_names=tuple(out_names),
                lowering_input_output_aliases=(),
                sim_require_finite=True,
                sim_require_nnan=True,
                nc=nc,
            )
            return tuple(outs)

        devices = jax.devices()[:E]
        mesh = Mesh(np.array(devices), ("core",))
        in_specs = (PartitionSpec("core"),) * (n_params + n_outs)
        out_specs = (PartitionSpec("core"),) * n_outs
        sharded = jax.jit(
            shard_map(_body, mesh=mesh, in_specs=in_specs,
                      out_specs=out_specs, check_rep=False),
            donate_argnums=donate, keep_unused=True,
        )
        _CACHED_RUNNER = (sharded, in_names, out_names, out_avals)

    sharded, in_names, out_names, out_avals = _CACHED_RUNNER
    concat_in = [
        np.concatenate([np.asarray(m[name]) for m in in_maps], axis=0)
        for name in in_names
    ]
    concat_zeros = [
        np.zeros((E * a.shape[0], *a.shape[1:]), a.dtype) for a in out_avals
    ]
    out_arrs = sharded(*concat_in, *concat_zeros)
    results = [
        {name: np.asarray(out_arrs[i]).reshape(E, *out_avals[i].shape)[c]
         for i, name in enumerate(out_names)}
        for c in range(E)
    ]
    return results


def combine(res, perm_idx):
    # (E, C, H) of scaled expert outputs, then scatter-add back to tokens.
    y_flat = np.empty((E * C, H), dtype=np.float32)
    for e in range(E):
        yt_e = res.results[e]["yt"].reshape(H, C)  # (H, C)
        y_flat[e * C:(e + 1) * C] = yt_e.T
    out = np.zeros((T, H), dtype=np.float32)
    for k in range(K):
        idx = perm_idx[:, k]
        m = idx > 0
        out[m] += y_flat[idx[m] - 1]
    return out


def kernel(hidden_states, expert_affinities, expert_index, gate_up_proj, down_proj):
    in_maps, perm_idx = prepare_in_maps(
        hidden_states, expert_affinities, expert_index, gate_up_proj, down_proj
    )
    try:
        results = _fast_run(in_maps)
    except Exception:
        results = run_spmd(in_maps).results
    return combine(_Res(results), perm_idx)


class _Res:
    def __init__(self, results):
        self.results = results


# revision 9
# speedup vs baseline: 1.1985x; 1.1977x over previous
"""MoE expert-MLP kernel for Trainium2, expert-parallel across 8 NeuronCores.

Problem: T=8192 tokens, H=1024 hidden, I=4096 intermediate, E=8 experts,
top-K=2, capacity C = T*K/E = 2048 slots per expert.

Strategy:
  - Host (numpy): replicate the reference routing exactly (capacity drop via
    token-order cumsum, top-k affinity renormalization), gather each expert's
    tokens into a dense (C, H) block, transpose to (H, C) so the contraction
    dim lands on SBUF partitions, and pre-tile the weights into lhsT layout.
  - Device (1 expert per core, same SPMD program): two grouped matmuls with
    fused SiLU-GLU, all in bf16 (fp32 PSUM accumulation).  bf16 weights get
    the fast-weight-load path (LDWEIGHTS fully hidden under the matmul),
    unlike fp32r whose 224ns weight loads cost ~13ns per matmul.  The
    per-slot combine scale is folded into the PSUM->SBUF evacuation of y^T
    (exact: scale commutes with the linear matmul).
  - Host: scatter-add y back to token order via the routing permutation.

C is processed in two halves of 1024 so h^T (bf16) fits in SBUF; W1 is
streamed twice (bf16: 2x16MB) which hides under the ~800us of PE work.
x^T lives in one 3D SBUF tile [128, KH, C] loaded chunk-major with few
large DMAs; the first k-tile of chunk 0 is a separate small DMA so the
very first matmul's dependencies land as early as possible (the sync
engine issues DMA descriptors at ~0.7us each, so issue ORDER, not
bandwidth, dominates time-to-first-matmul).
"""

import sys

import numpy as np

try:
    import concourse.bass as bass  # noqa: F401
except ImportError:
    for p in ("/opt/trn_rl_repo", "/root/.axon_site/_ro/trn_rl_repo"):
        if p not in sys.path:
            sys.path.insert(0, p)

import ml_dtypes
import concourse.bass as bass
import concourse.tile as tile
from concourse import bacc, mybir
from concourse.bass_utils import run_bass_kernel_spmd

dt = mybir.dt

T, H, I, E, K = 8192, 1024, 4096, 8, 2
C = 2048  # min(T, ceil(T*K*1.0/E))
KH = H // 128   # 8  k-tiles for the first contraction
KI = I // 128   # 32 k-tiles for the second contraction
M1 = 2 * I // 128  # 64 output tiles of gu^T
M2 = H // 128   # 8  output tiles of y^T
NCH = 2         # C halves
NW = 2          # 512-wide chunks per half
NQ = 4          # 512-wide chunks of C for x loading

_CACHED_NC = None


def _build_nc():
    nc = bacc.Bacc(None)
    xt = nc.dram_tensor("xt", [NQ, 128, KH, 512], dt.bfloat16, kind="ExternalInput")
    # w1 pairs the gate tile m and up tile m+M1/2 in one [128, 2048] row so a
    # single DMA fetches both per m-step.
    w1 = nc.dram_tensor("w1", [M1 // 2, 128, 2 * KH * 128], dt.bfloat16,
                        kind="ExternalInput")
    w2 = nc.dram_tensor("w2", [M2, 128, KI * 128], dt.bfloat16, kind="ExternalInput")
    sc = nc.dram_tensor("sc", [128, C], dt.float32, kind="ExternalInput")
    yt = nc.dram_tensor("yt", [M2, 128, C], dt.float32, kind="ExternalOutput")

    with tile.TileContext(nc) as tc:
        with (
            tc.tile_pool(name="xt_p", bufs=1) as xt_p,
            tc.tile_pool(name="sc_p", bufs=1) as sc_p,
            tc.tile_pool(name="ht_p", bufs=1) as ht_p,
            tc.tile_pool(name="w1_p", bufs=2) as w1_p,
            tc.tile_pool(name="w2_p", bufs=4) as w2_p,
            tc.tile_pool(name="sg_p", bufs=4) as sg_p,
            tc.tile_pool(name="yb_p", bufs=6) as yb_p,
            tc.tile_pool(name="ps", bufs=2, space="PSUM") as ps,
            tc.tile_pool(name="ps_y", bufs=4, space="PSUM") as ps_y,
        ):
            xt_t = xt_p.tile([128, KH, C], dt.bfloat16, tag="xt", name="xt_t")
            sc_t = sc_p.tile([128, C], dt.float32, tag="sc")

            def load_xt_chunk(q):
                nc.sync.dma_start(xt_t[:, :, q * 512:(q + 1) * 512], xt[q])

            # Head: two HWDGE issue streams in parallel.  Sync carries the x
            # chunk (k0/k1 first so matmul 0/1 can start, then the rest);
            # Scalar carries the first weight pair, sliced so the first
            # matmuls' weights land first.  The issuing engine needs ~0.65us
            # per DMA, so issue order — not bandwidth — sets
            # time-to-first-matmul.
            w1gu0 = w1_p.tile([128, 2 * KH * 128], dt.bfloat16, tag="w1gu",
                              bufs=6)
            nc.sync.dma_start(w1gu0[:, 0:256], w1[0][:, 0:256])
            nc.sync.dma_start(xt_t[:, 0:2, 0:512], xt[0][:, 0:2, :])
            nc.scalar.dma_start(w1gu0[:, 256:KH * 128], w1[0][:, 256:KH * 128])
            nc.scalar.dma_start(w1gu0[:, KH * 128:], w1[0][:, KH * 128:])
            nc.sync.dma_start(xt_t[:, 2:5, 0:512], xt[0][:, 2:5, :])
            nc.sync.dma_start(xt_t[:, 5:, 0:512], xt[0][:, 5:, :])
            load_xt_chunk(1)

            # First HEAD_M m-tiles run their n=0 window before any n=1 work:
            # the early DMA pipe can't feed both 512-col x chunks at the rate
            # the PE consumes them, so halve the early x working set.
            HEAD_M = 4

            def phase1_order():
                for mh in range(HEAD_M):
                    yield mh, 0
                for mh in range(HEAD_M):
                    yield mh, 1
                for mh in range(HEAD_M, M1 // 2):
                    yield mh, 0
                    yield mh, 1

            for ch in range(NCH):
                cw = C // NCH  # 1024
                # ---- phase 1: gu^T = W1^T x^T ; h^T = silu(g)*u (bf16) ----
                ht_t = {}
                w1gu_t = {}
                for mh, n in phase1_order():
                    if ch == 0 and n == 0:
                        if mh == 4:
                            load_xt_chunk(2)
                        if mh == 8:
                            load_xt_chunk(3)
                        if mh == 12:
                            nc.sync.dma_start(sc_t[:], sc[:])
                    if n == 0:
                        if ch == 0 and mh == 0:
                            w1gu = w1gu0
                        else:
                            w1gu = w1_p.tile([128, 2 * KH * 128], dt.bfloat16,
                                             tag="w1gu", bufs=6)
                            nc.scalar.dma_start(w1gu[:], w1[mh])
                        w1gu_t[mh] = w1gu
                        ht_t[mh] = ht_p.tile([128, cw], dt.bfloat16,
                                             tag=f"ht{mh}", name=f"ht{mh}")
                    w1gu = w1gu_t[mh]
                    ht = ht_t[mh]
                    w1g = w1gu[:, 0:KH * 128]
                    w1u = w1gu[:, KH * 128:]
                    col = ch * cw + n * 512
                    pg = ps.tile([128, 512], dt.float32, tag="pg")
                    pu = ps.tile([128, 512], dt.float32, tag="pu")
                    for k in range(KH):
                        nc.tensor.matmul(
                            pg[:],
                            w1g[:, k * 128:(k + 1) * 128],
                            xt_t[:, k, col:col + 512],
                            start=(k == 0),
                            stop=(k == KH - 1),
                        )
                    for k in range(KH):
                        nc.tensor.matmul(
                            pu[:],
                            w1u[:, k * 128:(k + 1) * 128],
                            xt_t[:, k, col:col + 512],
                            start=(k == 0),
                            stop=(k == KH - 1),
                        )
                    sg = sg_p.tile([128, 512], dt.float32, tag="sg")
                    nc.scalar.activation(
                        sg[:], pg[:], mybir.ActivationFunctionType.Silu
                    )
                    nc.vector.tensor_mul(
                        ht[:, n * 512:(n + 1) * 512], sg[:], pu[:]
                    )

                # ---- phase 2: y^T = W2^T h^T, scaled on evacuation ----
                for m in range(M2):
                    w2t = w2_p.tile([128, KI * 128], dt.bfloat16, tag="w2")
                    nc.sync.dma_start(w2t[:], w2[m])
                    for n in range(NW):
                        col = ch * cw + n * 512
                        # The very last output tile drains in 256-col halves
                        # so its evacuation+writeback overlaps the final
                        # matmuls instead of all trailing them.
                        nsplit = 2 if (ch == NCH - 1 and m == M2 - 1
                                       and n == NW - 1) else 1
                        w = 512 // nsplit
                        for hf in range(nsplit):
                            c2 = col + hf * w
                            off = n * 512 + hf * w
                            py = ps_y.tile([128, w], dt.float32, tag="py")
                            for k in range(KI):
                                nc.tensor.matmul(
                                    py[:],
                                    w2t[:, k * 128:(k + 1) * 128],
                                    ht_t[k][:, off:off + w],
                                    start=(k == 0),
                                    stop=(k == KI - 1),
                                )
                            yb = yb_p.tile([128, w], dt.float32, tag="yb")
                            nc.vector.tensor_mul(yb[:], py[:],
                                                 sc_t[:, c2:c2 + w])
                            nc.sync.dma_start(yt[m][:, c2:c2 + w], yb[:])
    nc.finalize()
    return nc


def _route(expert_affinities, expert_index):
    """Numpy port of the reference routing. Returns (tok_idx, valid, scale)."""
    mask = np.zeros((T, E), dtype=np.float32)
    rows = np.arange(T)[:, None]
    mask[rows, expert_index] = 1.0  # top-k entries are distinct per token
    position = np.cumsum(mask, axis=0, dtype=np.float32)  # 1-based
    mask = np.where(position > C, 0.0, mask)

    affin = np.where(mask == 0, 0.0, expert_affinities)
    denom = np.maximum(np.sum(np.abs(affin), axis=1, keepdims=True), 1e-12)
    affin = affin / denom

    offsets = np.arange(E, dtype=np.float32) * C
    pos_off = np.where(mask == 0, 0.0, position + offsets)
    perm_idx = np.take_along_axis(pos_off, expert_index, axis=1).astype(np.int32)

    tok_ids = np.broadcast_to(
        np.arange(1, T + 1, dtype=np.int32)[:, None], (T, K)
    )
    assignments = np.zeros(E * C + 1, dtype=np.int32)
    assignments[perm_idx.reshape(-1)] = tok_ids.reshape(-1)
    assignments = assignments[1:].reshape(E, C) - 1
    valid = assignments >= 0
    tok_idx = np.maximum(assignments, 0)

    scale = affin[tok_idx, np.arange(E)[:, None]] * valid.astype(np.float32)
    return tok_idx, valid, scale, perm_idx


def prepare_in_maps(hidden_states, expert_affinities, expert_index,
                    gate_up_proj, down_proj):
    hidden_states = np.asarray(hidden_states, dtype=np.float32)
    expert_affinities = np.asarray(expert_affinities, dtype=np.float32)
    expert_index = np.asarray(expert_index, dtype=np.int32)
    gate_up_proj = np.asarray(gate_up_proj, dtype=np.float32)
    down_proj = np.asarray(down_proj, dtype=np.float32)

    tok_idx, valid, scale, perm_idx = _route(expert_affinities, expert_index)

    def _prep_expert(e):
        x_e = hidden_states[tok_idx[e]]  # (C, H)
        xt_e = np.ascontiguousarray(x_e.T).astype(ml_dtypes.bfloat16)  # (H, C)
        xt_e = np.ascontiguousarray(
            xt_e.reshape(KH, 128, NQ, 512).transpose(2, 1, 0, 3)
        )  # (NQ, 128, KH, 512)
        # (m, p, pair, k, j): pair 0 = gate column block m, pair 1 = up block m
        w1_e = np.ascontiguousarray(
            gate_up_proj[e].astype(ml_dtypes.bfloat16)
            .reshape(KH, 128, 2, M1 // 2, 128).transpose(3, 1, 2, 0, 4)
        ).reshape(M1 // 2, 128, 2 * KH * 128)
        w2_e = np.ascontiguousarray(
            down_proj[e].astype(ml_dtypes.bfloat16)
            .reshape(KI, 128, M2, 128).transpose(2, 1, 0, 3)
        ).reshape(M2, 128, KI * 128)
        sc_e = np.broadcast_to(scale[e][None, :], (128, C)).copy()
        return {"xt": xt_e, "w1": w1_e, "w2": w2_e, "sc": sc_e}

    from concurrent.futures import ThreadPoolExecutor
    with ThreadPoolExecutor(max_workers=E) as pool:
        in_maps = list(pool.map(_prep_expert, range(E)))
    return in_maps, perm_idx


def run_spmd(in_maps, **kwargs):
    global _CACHED_NC
    if _CACHED_NC is None:
        _CACHED_NC = _build_nc()
    return run_bass_kernel_spmd(
        _CACHED_NC, in_maps, core_ids=list(range(E)), **kwargs
    )


_CACHED_RUNNER = None


def _fast_run(in_maps):
    """Same semantics as run_bass_kernel_spmd under axon, but the jitted
    shard_map callable is built once and reused, avoiding per-call retrace."""
    global _CACHED_NC, _CACHED_RUNNER
    if _CACHED_RUNNER is None:
        if _CACHED_NC is None:
            _CACHED_NC = _build_nc()
        nc = _CACHED_NC
        import jax
        from jax.sharding import Mesh, PartitionSpec
        try:
            from jax.experimental.shard_map import shard_map
        except ImportError:
            from jax.shard_map import shard_map  # newer jax
        from concourse import bass2jax, mybir as _mybir
        bass2jax.install_neuronx_cc_hook()

        partition_name = (
            nc.partition_id_tensor.name if nc.partition_id_tensor else None
        )
        in_names, out_names, out_avals = [], [], []
        for alloc in nc.m.functions[0].allocations:
            if not isinstance(alloc, _mybir.MemoryLocationSet):
                continue
            name = alloc.memorylocations[0].name
            if alloc.kind == "ExternalInput":
                if name != partition_name:
                    in_names.append(name)
            elif alloc.kind == "ExternalOutput":
                out_names.append(name)
                out_avals.append(jax.core.ShapedArray(
                    tuple(alloc.tensor_shape), _mybir.dt.np(alloc.dtype)))
        n_params = len(in_names)
        n_outs = len(out_avals)
        all_in_names = list(in_names) + list(out_names)
        if partition_name is not None:
            all_in_names.append(partition_name)
        donate = tuple(range(n_params, n_params + n_outs))

        def _body(*args):
            operands = list(args)
            if partition_name is not None:
                operands.append(bass2jax.partition_id_tensor())
            outs = bass2jax._bass_exec_p.bind(
                *operands,
                out_avals=tuple(out_avals),
                in_names=tuple(all_in_names),
                out_names=tuple(out_names),
                lowering_input_output_aliases=(),
                sim_require_finite=True,
                sim_require_nnan=True,
                nc=nc,
            )
            return tuple(outs)

        devices = jax.devices()[:E]
        mesh = Mesh(np.array(devices), ("core",))
        in_specs = (PartitionSpec("core"),) * (n_params + n_outs)
        out_specs = (PartitionSpec("core"),) * n_outs
        sharded = jax.jit(
            shard_map(_body, mesh=mesh, in_specs=in_specs,
                      out_specs=out_specs, check_rep=False),
            donate_argnums=donate, keep_unused=True,
        )
        _CACHED_RUNNER = (sharded, in_names, out_names, out_avals)

    sharded, in_names, out_names, out_avals = _CACHED_RUNNER
    concat_in = [
        np.concatenate([np.asarray(m[name]) for m in in_maps], axis=0)
        for name in in_names
    ]
    concat_zeros = [
        np.zeros((E * a.shape[0], *a.shape[1:]), a.dtype) for a in out_avals
    ]
    out_arrs = sharded(*concat_in, *concat_zeros)
    results = [
        {name: np.asarray(out_arrs[i]).reshape(E, *out_avals[i].shape)[c]
         for i, name in enumerate(out_names)}
        for c in range(E)
    ]
    return results


def combine(res, perm_idx):
    # (E, C, H) of scaled expert outputs, then scatter-add back to tokens.
    y_flat = np.empty((E * C, H), dtype=np.float32)
    for e in range(E):
        yt_e = res.results[e]["yt"].reshape(H, C)  # (H, C)
        y_flat[e * C:(e + 1) * C] = yt_e.T
    out = np.zeros((T, H), dtype=np.float32)
    for k in range(K):
        idx = perm_idx[:, k]
        m = idx > 0
        out[m] += y_flat[idx[m] - 1]
    return out


def kernel(hidden_states, expert_affinities, expert_index, gate_up_proj, down_proj):
    in_maps, perm_idx = prepare_in_maps(
        hidden_states, expert_affinities, expert_index, gate_up_proj, down_proj
    )
    try:
        results = _fast_run(in_maps)
    except Exception:
        results = run_spmd(in_maps).results
    return combine(_Res(results), perm_idx)


class _Res:
    def __init__(self, results):
        self.results = results
